# revision 1
# baseline (speedup 1.0000x reference)
"""Causal GQA self-attention on 8 Trainium2 NeuronCores.

Sharding: data-parallel over batch (4) x tensor-parallel over heads (2 halves
of 14 heads each, KV heads replicated for the shared GQA group). Each core
computes a partial output (its heads' contribution through the row-parallel
out-projection); the host sums the two partials per batch element.

Per-core head assignment is chosen so every core sees an identical local
structure (local heads 0..13, local kv-groups 0..3, quad q <-> group q):
  half 0: global heads [0..11, 24, 25],  kv heads [0, 1, 2, 6]
  half 1: global heads [12..23, 26, 27], kv heads [3, 4, 5, 6]
The host permutes weight columns/rows into this local order.

Kernel layout strategy (all SBUF tensors [128 partitions, free...]):
  xT  [128, 7, 2048] : x^T (C on partitions) via PE transpose
  QT  [128, 4, 2048] : Q^T, local head h at (partitions 32*(h%4), chunk h//4)
  KT  [128, 4, 2048] : K^T per local group, replicated on all 4 row slots
  V   [128, 16, 128] : V (kpos on partitions)
  AOT [128, 4, 2048] : attention output transposed (head dims on partitions)
Scores are computed transposed S^T[kpos, q] with 4 row-tiled (tile_position)
K=32 matmuls per quad; exp on ScalarE (PSUM->SBUF, scale folded in); P^T then
feeds col-tiled AV and Z(=sum) matmuls accumulating over kpos chunks; final
out-projection consumes AOT directly as the stationary operand.
"""

import sys

sys.path.insert(0, "/opt/trn_rl_repo")

from contextlib import ExitStack

import numpy as np

import concourse.bass as bass
import concourse.mybir as mybir
import concourse.tile as tile
from concourse import bacc
from concourse.bass import ts
from concourse.bass_utils import run_bass_kernel_spmd

F32 = mybir.dt.float32
F32R = mybir.dt.float32r
EXP = mybir.ActivationFunctionType.Exp
P = 128
T, C = 2048, 896
D = 32
HL = 14  # local heads per core
GL = 4  # local kv groups per core
DH = HL * D  # 448
DKV = GL * D  # 128
SCALE = 1.0 / float(np.sqrt(D))

HEADS_HALF = [
    list(range(0, 12)) + [24, 25],
    list(range(12, 24)) + [26, 27],
]
KV_HALF = [[0, 1, 2, 6], [3, 4, 5, 6]]


def _trace(tc, d):
    nc = tc.nc
    with ExitStack() as ctx:
        const = ctx.enter_context(tc.tile_pool(name="const", bufs=1))
        ident = const.tile([P, P], F32)
        nc.sync.dma_start(ident[:], d["ident"][:])
        maskb = const.tile([P, P], F32)
        nc.sync.dma_start(maskb[:], d["mask"][:])
        identr = const.tile([P, P], F32R)
        nc.sync.dma_start(identr[:], d["identr"][:])

        persist = ctx.enter_context(tc.tile_pool(name="persist", bufs=1))
        QT = persist.tile([P, 4, T], F32R, tag="QT")
        KT = persist.tile([P, 4, T], F32R, tag="KT")
        V = persist.tile([P, 16, GL, 64], F32R, tag="V")

        nc.sync.dma_start(
            V[:, :, :, D:64],
            d["vones"].rearrange("p (a b c) -> p a b c", a=16, b=GL),
        )

        with tc.tile_pool(name="ph01", bufs=1) as ph01:
            xT = ph01.tile([P, 7, T], F32R, tag="xT")
            # ------------- phase 0: x -> xT (PE transpose) -------------
            with tc.tile_pool(name="xraw", bufs=8) as xraw, \
                 tc.tile_pool(name="pst", bufs=2, space="PSUM") as pst:
                xv = d["x"].rearrange("(to ti) c -> ti to c", ti=P)
                for tcg in range(4):
                    xt4 = []
                    for k in range(4):
                        xtile = xraw.tile([P, C], F32, tag="xtile")
                        nc.sync.dma_start(xtile[:], xv[:, 4 * tcg + k, :])
                        xt4.append(xtile)
                    for cc in range(7):
                        ps = pst.tile([P, 512], F32, tag="tps")
                        for k in range(4):
                            nc.tensor.transpose(
                                ps[:, ts(k, P)], xt4[k][:, ts(cc, P)], ident[:]
                            )
                        nc.vector.tensor_copy(xT[:, cc, ts(tcg, 512)], ps[:])

            # ---------------- phase 1: projections ----------------
            with tc.tile_pool(name="w1", bufs=1) as w1, \
                 tc.tile_pool(name="vtt", bufs=2) as vtt, \
                 tc.tile_pool(name="pst2", bufs=2, space="PSUM") as pst2, \
                 tc.tile_pool(name="psp", bufs=2, space="PSUM") as psp:
                WqH = w1.tile([P, 7, DH], F32R, tag="WqH")
                nc.sync.dma_start(
                    WqH[:], d["wq"].rearrange("(co ci) n -> ci co n", ci=P)
                )
                WkR = w1.tile([P, 7, GL, P], F32R, tag="WkR")
                wkv = d["wk"].rearrange("(co ci) n -> ci co n", ci=P)
                for g in range(GL):
                    for i in range(4):
                        nc.sync.dma_start(
                            WkR[:, :, g, ts(i, D)], wkv[:, :, ts(g, D)]
                        )
                WvH = w1.tile([P, 7, DKV], F32R, tag="WvH")
                nc.sync.dma_start(
                    WvH[:], d["wv"].rearrange("(co ci) n -> ci co n", ci=P)
                )

                # QT: out[m=dim chunk, n=t] accumulate over C chunks
                for mc in range(4):
                    M = P if mc < 3 else 64
                    for nk in range(4):
                        ps = psp.tile([P, 512], F32, tag="pps")
                        for c in range(7):
                            nc.tensor.matmul(
                                ps[:M, :],
                                lhsT=WqH[:, c, mc * P : mc * P + M],
                                rhs=xT[:, c, ts(nk, 512)],
                                start=(c == 0),
                                stop=(c == 6),
                            )
                        nc.vector.tensor_copy(QT[:M, mc, ts(nk, 512)], ps[:M, :])
                # KT (replicated): per local group
                for g in range(GL):
                    for nk in range(4):
                        ps = psp.tile([P, 512], F32, tag="pps")
                        for c in range(7):
                            nc.tensor.matmul(
                                ps[:],
                                lhsT=WkR[:, c, g, :],
                                rhs=xT[:, c, ts(nk, 512)],
                                start=(c == 0),
                                stop=(c == 6),
                            )
                        nc.vector.tensor_copy(KT[:, g, ts(nk, 512)], ps[:])
                # VT then transpose to V
                for nk in range(4):
                    ps = psp.tile([P, 512], F32, tag="pps")
                    for c in range(7):
                        nc.tensor.matmul(
                            ps[:],
                            lhsT=WvH[:, c, :],
                            rhs=xT[:, c, ts(nk, 512)],
                            start=(c == 0),
                            stop=(c == 6),
                        )
                    vts = vtt.tile([P, 512], F32, tag="vts")
                    nc.vector.tensor_copy(vts[:], ps[:])
                    for k in range(4):
                        vps = pst2.tile([P, 512], F32, tag="tps")
                        nc.tensor.transpose(vps[:, :P], vts[:, ts(k, P)], ident[:])
                        nc.vector.tensor_copy(
                            V[:, nk * 4 + k, :, 0:D],
                            vps[:, :P].rearrange("p (g e) -> p g e", g=GL),
                        )

        # ---------------- phase 2+3: attention + out-proj ----------------
        with tc.tile_pool(name="w2", bufs=1) as w2, \
             tc.tile_pool(name="pts", bufs=2) as pts, \
             tc.tile_pool(name="ziP", bufs=2) as zip_, \
             tc.tile_pool(name="outs", bufs=2) as outs_p, \
             tc.tile_pool(name="pss", bufs=2, space="PSUM") as pss, \
             tc.tile_pool(name="psav", bufs=2, space="PSUM") as psav, \
             tc.tile_pool(name="pso", bufs=2, space="PSUM") as pso:
            AOT = w2.tile([P, 4, T], F32R, tag="AOT")
            WoH = w2.tile([P, 4, C], F32R, tag="WoH")
            nc.sync.dma_start(
                WoH[:, :3, :], d["wo"][: 3 * P, :].rearrange("(co ci) n -> ci co n", ci=P)
            )
            nc.sync.dma_start(WoH[:64, 3, :], d["wo"][3 * P :, :])
            ov = d["out"].rearrange("(to ti) c -> ti to c", ti=P)

            for qc in range(4):
                qs = qc * 512
                for pr in range(7):
                    h0 = 2 * pr
                    g = h0 // 4
                    j0 = h0 % 4
                    ava = psav.tile([64, 512], F32, tag="av")
                    avb = psav.tile([64, 512], F32, tag="av")
                    avs = [ava, avb]
                    nks = qs // P + 4
                    for ki in range(nks):
                        ks = ki * P
                        qoff = max(0, ks - qs)
                        pt = pts.tile([P, 2, 512], F32R, tag="pt")
                        sp = pss.tile([P, 2, 512], F32, tag="sp")
                        for j2 in range(2):
                            j = j0 + j2
                            nc.tensor.matmul(
                                sp[:, j2, qoff:512],
                                lhsT=KT[ts(j, D), g, ks : ks + P],
                                rhs=QT[ts(j, D), g, qs + qoff : qs + 512],
                                start=True,
                                stop=True,
                                tile_position=(j * D, 0),
                            )
                        nc.scalar.activation(
                            pt[:, :, qoff:512],
                            sp[:, :, qoff:512],
                            EXP,
                            scale=SCALE,
                        )
                        if ks >= qs:  # diagonal chunk: zero the triangle
                            nc.vector.tensor_tensor(
                                pt[:, :, qoff : qoff + P],
                                pt[:, :, qoff : qoff + P],
                                maskb[:, None, :].to_broadcast((P, 2, P)),
                                mybir.AluOpType.mult,
                            )
                        for j2 in range(2):
                            nc.tensor.matmul(
                                avs[j2][0:64, qoff:512],
                                lhsT=V[:, ki, g, 0:64],
                                rhs=pt[:, j2, qoff:512],
                                start=(ki == 0),
                                stop=(ki == nks - 1),
                                skip_group_check=True,
                            )
                    zq = pss.tile([P, 2, 512], F32, tag="sp")
                    for j2 in range(2):
                        h = h0 + j2
                        av = avs[j2]
                        zt = zip_.tile([64, 512], F32R, tag="zt")
                        nc.vector.tensor_copy(zt[D:64, :], av[D:64, :])
                        nc.tensor.matmul(
                            zq[0:D, j2, :],
                            lhsT=identr[D:64, D:64],
                            rhs=zt[D:64, :],
                            start=True,
                            stop=True,
                            tile_position=(D, 0),
                        )
                        zs = zip_.tile([D, 512], F32, tag="zs")
                        nc.vector.reciprocal_approx_fast(zs[:], zq[0:D, j2, :])
                        ao = zip_.tile([D, 512], F32R, tag="ao")
                        nc.vector.tensor_tensor(
                            ao[:],
                            av[0:D, :],
                            zs[:],
                            mybir.AluOpType.mult,
                        )
                        nc.sync.dma_start(
                            AOT[ts(h % 4, D), g, qs : qs + 512], ao[:]
                        )
                # out-projection for this q-chunk
                for tcl in range(4):
                    tg = qc * 4 + tcl
                    ob = outs_p.tile([P, C], F32, tag="ob")
                    for ncol in range(2):
                        po = pso.tile([P, 448], F32, tag="po")
                        for c in range(4):
                            K = P if c < 3 else 64
                            nc.tensor.matmul(
                                po[:],
                                lhsT=AOT[:K, c, qs + tcl * P : qs + (tcl + 1) * P],
                                rhs=WoH[:K, c, ncol * 448 : (ncol + 1) * 448],
                                start=(c == 0),
                                stop=(c == 3),
                            )
                        nc.vector.tensor_copy(ob[:, ncol * 448 : (ncol + 1) * 448], po[:])
                    nc.sync.dma_start(ov[:, tg, :], ob[:])


_NC_CACHE = None


def _build():
    global _NC_CACHE
    if _NC_CACHE is not None:
        return _NC_CACHE
    nc = bacc.Bacc("TRN2", target_bir_lowering=False, debug=False, num_devices=8)
    d = {
        "x": nc.dram_tensor("x", (T, C), F32, kind="ExternalInput"),
        "wq": nc.dram_tensor("wq", (C, DH), F32R, kind="ExternalInput"),
        "wk": nc.dram_tensor("wk", (C, DKV), F32R, kind="ExternalInput"),
        "wv": nc.dram_tensor("wv", (C, DKV), F32R, kind="ExternalInput"),
        "wo": nc.dram_tensor("wo", (DH, C), F32R, kind="ExternalInput"),
        "ident": nc.dram_tensor("ident", (P, P), F32, kind="ExternalInput"),
        "mask": nc.dram_tensor("mask", (P, P), F32, kind="ExternalInput"),
        "vones": nc.dram_tensor("vones", (P, 16 * GL * D), F32R, kind="ExternalInput"),
        "identr": nc.dram_tensor("identr", (P, P), F32R, kind="ExternalInput"),
        "out": nc.dram_tensor("out", (T, C), F32, kind="ExternalOutput"),

    }
    with tile.TileContext(nc) as tc:
        _trace(tc, {k: v[:] for k, v in d.items()})
    nc.compile()
    _NC_CACHE = nc
    return nc


def _in_maps(x, Wq, Wk, Wv, Wo):
    ident = np.eye(P, dtype=np.float32)
    vones = np.ones((P, 16 * GL * D), dtype=np.float32)
    maskb = (
        np.arange(P)[None, :] >= np.arange(P)[:, None]
    ).astype(np.float32)  # [kpos_p, q_j] valid when j >= p
    maps = []
    for c in range(8):
        b, hf = c // 2, c % 2
        hcols = np.concatenate([np.arange(32 * h, 32 * h + 32) for h in HEADS_HALF[hf]])
        kcols = np.concatenate([np.arange(32 * g, 32 * g + 32) for g in KV_HALF[hf]])
        maps.append(
            {
                "x": np.ascontiguousarray(x[b]),
                "wq": np.ascontiguousarray(Wq[:, hcols]),
                "wk": np.ascontiguousarray(Wk[:, kcols]),
                "wv": np.ascontiguousarray(Wv[:, kcols]),
                "wo": np.ascontiguousarray(Wo[hcols, :]),
                "ident": ident,
                "mask": maskb,
                "vones": vones,
                "identr": ident,
            }
        )
    return maps


def run(x, Wq, Wk, Wv, Wo, trace=False):
    nc = _build()
    res = run_bass_kernel_spmd(
        nc, _in_maps(x, Wq, Wk, Wv, Wo), core_ids=list(range(8)), trace=trace
    )
    outs = [r["out"] for r in res.results]
    final = np.empty((4, T, C), np.float32)
    for b in range(4):
        final[b] = outs[2 * b] + outs[2 * b + 1]
    return final, res


def kernel(x, Wq, Wk, Wv, Wo):
    x = np.asarray(x, dtype=np.float32)
    out, _ = run(
        x,
        np.asarray(Wq, np.float32),
        np.asarray(Wk, np.float32),
        np.asarray(Wv, np.float32),
        np.asarray(Wo, np.float32),
    )
    return out



# revision 20
# speedup vs baseline: 1.3316x; 1.3316x over previous
"""Causal GQA self-attention on 8 Trainium2 NeuronCores.

Sharding: data-parallel over batch (4) x tensor-parallel over heads (2 halves
of 14 heads each, KV heads replicated for the shared GQA group). Each core
computes a partial output (its heads' contribution through the row-parallel
out-projection); the host sums the two partials per batch element.

Local structure per core: 14 local heads, 4 local kv-groups, head l ->
group l//4, slot l%4 (group 3 has only heads 12,13 = slots 0,1).

Kernel layout (partition dim first):
  xT   [128, 7, 2048]     bf16 : x^T (C on partitions)
  QT8  [128, 4, 2, 2048]  e4m3 : band g (32 rows) of slot s = head 4g+s,
                                 duplicated on both DoubleRow k-tiles
  KT8  [128, 2, 2048]     e4m3 : band g = group g; k-tile 0 = hi, 1 = lo
                                 (hi/lo split recovers bf16-level precision)
  V    [128, 16, 4, 33]   bf16 : kpos on partitions, per (kchunk, group);
                                 col 32 = ones (gives softmax denominator Z)
  PT   [128, 15, 512]     bf16 : exp(scores^T) per (head, qc)
Scores are computed transposed S^T[kpos, q] with fp8 DoubleRow matmuls
(0.5 cyc/row); exp on ScalarE in 3-bank batches (scale folded in); AV uses
P^T chunks as the stationary operand -> out[q, 33] (N=33 per matmul);
division by Z is a per-partition broadcast divide on GPSIMD; transposed
AO quads feed the out-projection.
"""

import sys

sys.path.insert(0, "/opt/trn_rl_repo")

from contextlib import ExitStack

import numpy as np
import ml_dtypes

import concourse.bass as bass
import concourse.mybir as mybir
import concourse.tile as tile
from concourse import bacc
from concourse.bass import ts
from concourse.bass_utils import run_bass_kernel_spmd

F32 = mybir.dt.float32
BF16 = mybir.dt.bfloat16
F8 = mybir.dt.float8e4
EXP = mybir.ActivationFunctionType.Exp
DR = mybir.MatmulPerfMode.DoubleRow
MUL = mybir.AluOpType.mult
SUB = mybir.AluOpType.subtract
DIV = mybir.AluOpType.divide
BYP = mybir.AluOpType.bypass

P = 128
T, C = 2048, 896
D = 32
HL = 14  # local heads per core
GL = 4  # local kv groups per core
DH = HL * D  # 448
SCALE = 1.0 / float(np.sqrt(D))
NK = T // 512  # 4 q-chunks of 512
NT = T // P  # 16 kpos chunks / t tiles

HEADS_HALF = [
    list(range(0, 12)) + [24, 25],
    list(range(12, 24)) + [26, 27],
]
KV_HALF = [[0, 1, 2, 6], [3, 4, 5, 6]]


def _phase_a(tc, d, ctx):
    """x transpose + projections. Returns persistent SBUF tiles."""
    nc = tc.nc
    const = ctx.enter_context(tc.tile_pool(name="const", bufs=1))
    ident = const.tile([P, P], BF16)
    nc.sync.dma_start(ident[:], d["ident"][:])
    identr = const.tile([P, P], mybir.dt.float32r)
    nc.sync.dma_start(identr[:], d["identr"][:])
    maskb = const.tile([P, P], BF16)
    nc.sync.dma_start(maskb[:], d["mask"][:])
    zlhs = const.tile([P, 2, P], F8)
    nc.vector.memset(zlhs[:], 0.0)

    persist = ctx.enter_context(tc.tile_pool(name="persist", bufs=1))
    QT8 = persist.tile([P, 4, 2, T], F8, tag="QT8")
    KT8 = persist.tile([P, 2, T], F8, tag="KT8")
    V = persist.tile([P, NT, GL, 33], BF16, tag="V")
    nc.vector.memset(V[:, :, :, 32:33], 1.0)

    with tc.tile_pool(name="ph01", bufs=1) as ph01:
        xT = ph01.tile([P, 7, T], BF16, tag="xT")
        # ---- phase 0: x -> xT (PE transpose, bf16 identity = 1 cyc/row) ----
        with tc.tile_pool(name="xraw", bufs=8) as xraw, \
             tc.tile_pool(name="pst", bufs=2, space="PSUM") as pst:
            xv = d["x"].rearrange("(to ti) c -> ti to c", ti=P)
            for tcg in range(4):
                xt4 = []
                for k in range(4):
                    xtile = xraw.tile([P, C], mybir.dt.float32r, tag="xtile")
                    nc.sync.dma_start(xtile[:], xv[:, 4 * tcg + k, :])
                    xt4.append(xtile)
                for cc in range(7):
                    ps = pst.tile([P, 512], mybir.dt.float32r, tag="tps")
                    for k in range(4):
                        nc.tensor.transpose(
                            ps[:, ts(k, P)], xt4[k][:, ts(cc, P)], identr[:]
                        )
                    nc.vector.tensor_copy(xT[:, cc, ts(tcg, 512)], ps[:])

        # ---- phase 1: projections ----
        with tc.tile_pool(name="w1", bufs=1) as w1, \
             tc.tile_pool(name="vtt", bufs=1) as vtt, \
             tc.tile_pool(name="pst2", bufs=2, space="PSUM") as pst2, \
             tc.tile_pool(name="psp", bufs=2, space="PSUM") as psp:
            WqS = w1.tile([P, 7, 512], BF16, tag="WqS")
            nc.sync.dma_start(
                WqS[:], d["wq"].rearrange("(co ci) n -> ci co n", ci=P)
            )
            WkS = w1.tile([P, 7, P], BF16, tag="WkS")
            nc.sync.dma_start(
                WkS[:], d["wk"].rearrange("(co ci) n -> ci co n", ci=P)
            )
            WvS = w1.tile([P, 7, P], BF16, tag="WvS")
            nc.sync.dma_start(
                WvS[:], d["wv"].rearrange("(co ci) n -> ci co n", ci=P)
            )

            # Q: per slot s, band g holds head 4g+s; duplicate on both k-tiles
            for s in range(4):
                for nk in range(NK):
                    ps = psp.tile([P, 512], F32, tag="pps")
                    for c in range(7):
                        nc.tensor.matmul(
                            ps[:],
                            lhsT=WqS[:, c, ts(s, P)],
                            rhs=xT[:, c, ts(nk, 512)],
                            start=(c == 0),
                            stop=(c == 6),
                        )
                    nc.vector.tensor_copy(
                        QT8[:, s, :, ts(nk, 512)],
                        ps[:, None, :].to_broadcast((P, 2, 512)),
                    )
            # K: hi/lo split across the two DoubleRow k-tiles
            for nk in range(NK):
                ps = psp.tile([P, 512], F32, tag="pps")
                for c in range(7):
                    nc.tensor.matmul(
                        ps[:],
                        lhsT=WkS[:, c, :],
                        rhs=xT[:, c, ts(nk, 512)],
                        start=(c == 0),
                        stop=(c == 6),
                    )
                nc.vector.tensor_copy(KT8[:, 0, ts(nk, 512)], ps[:])
                nc.vector.tensor_tensor(
                    KT8[:, 1, ts(nk, 512)], ps[:], KT8[:, 0, ts(nk, 512)], SUB
                )
            # V: project then transpose to [kpos, d]
            VTs = vtt.tile([P, T], BF16, tag="VTs")
            for nk in range(NK):
                ps = psp.tile([P, 512], F32, tag="pps")
                for c in range(7):
                    nc.tensor.matmul(
                        ps[:],
                        lhsT=WvS[:, c, :],
                        rhs=xT[:, c, ts(nk, 512)],
                        start=(c == 0),
                        stop=(c == 6),
                    )
                nc.vector.tensor_copy(VTs[:, ts(nk, 512)], ps[:])
            for k in range(NT):
                vps = pst2.tile([P, P], BF16, tag="vtp")
                nc.tensor.transpose(vps[:], VTs[:, ts(k, P)], ident[:])
                nc.vector.tensor_copy(
                    V[:, k, :, 0:D],
                    vps[:].rearrange("p (g e) -> p g e", g=GL),
                )
    return ident, maskb, zlhs, QT8, KT8, V


def _trace(tc, d):
    nc = tc.nc
    with ExitStack() as ctx:
        ident, maskb, zlhs, QT8, KT8, V = _phase_a(tc, d, ctx)

        # ---- phase 2: attention + out-projection, software pipelined ----
        w2 = ctx.enter_context(tc.tile_pool(name="w2", bufs=1))
        WoS = w2.tile([P, 4, C], BF16, tag="WoS")
        nc.sync.dma_start(
            WoS[:, :3, :], d["wo"][: 3 * P, :].rearrange("(co ci) n -> ci co n", ci=P)
        )
        nc.sync.dma_start(WoS[:64, 3, :], d["wo"][3 * P :, :])
        ov = d["out"].rearrange("(to ti) c -> ti to c", ti=P)

        sp_pool = ctx.enter_context(tc.tile_pool(name="spp", bufs=2, space="PSUM"))
        av_pool = ctx.enter_context(tc.tile_pool(name="avp", bufs=2, space="PSUM"))
        pt_pool = ctx.enter_context(tc.tile_pool(name="ptp", bufs=2))
        ao_pool = ctx.enter_context(tc.tile_pool(name="aop", bufs=2))
        ob_pool = ctx.enter_context(tc.tile_pool(name="obp", bufs=2))
        zr_pool = ctx.enter_context(tc.tile_pool(name="zrp", bufs=2))

        def emit_scores(qc, h):
            """scores (fp8 DoubleRow) + exp for one (qc, head). Returns PT."""
            g, s = h // 4, h % 4
            qs = qc * 512
            b = 32 * g
            nf = 4 * qc  # full kpos chunks
            PT = pt_pool.tile([P, 15, 512], BF16, tag="pt")
            rhs_full = QT8[b : b + D, s, :, qs : qs + 512]
            for kb in range(0, nf, 3):
                nb = min(3, nf - kb)
                sp = sp_pool.tile([P, 3, 512], F32, tag="sp")
                for i in range(nb):
                    ks = (kb + i) * P
                    nc.tensor.matmul(
                        sp[:, i, :],
                        lhsT=KT8[b : b + D, :, ks : ks + P],
                        rhs=rhs_full,
                        start=True,
                        stop=True,
                        perf_mode=DR,
                        tile_position=(b, 0),
                    )
                nc.scalar.activation(
                    PT[:, kb : kb + nb, :], sp[:, 0:nb, :], EXP, scale=SCALE
                )
            # diagonal set: chunks nf..nf+3 packed into 3 banks
            # D0 = chunk nf (span 512); D1 = chunk nf+1 (384) + nf+3 (128);
            # D2 = chunk nf+2 (256) + zero fill
            spd = sp_pool.tile([P, 3, 512], F32, tag="sp")
            km = [
                (0, 0, 0, 512),  # di, bank, col, span
                (1, 1, 0, 384),
                (2, 2, 0, 256),
                (3, 1, 384, 128),
            ]
            for di, bank, col, span in km:
                ks = qs + di * P
                nc.tensor.matmul(
                    spd[:, bank, col : col + span],
                    lhsT=KT8[b : b + D, :, ks : ks + P],
                    rhs=QT8[b : b + D, s, :, ks : ks + span],
                    start=True,
                    stop=True,
                    perf_mode=DR,
                    tile_position=(b, 0),
                )
            nc.tensor.matmul(
                spd[:, 2, 256:512],
                lhsT=zlhs[b : b + D, :, :],
                rhs=QT8[b : b + D, s, :, 0:256],
                start=True,
                stop=True,
                perf_mode=DR,
                tile_position=(b, 0),
            )
            nc.scalar.activation(
                PT[:, nf : nf + 3, :], spd[:], EXP, scale=SCALE
            )
            return PT

        def emit_av(qc, h, PT, AO):
            """mask + AV + division epilogue for one (qc, head)."""
            g = h // 4
            nf = 4 * qc
            # mask diag triangles: chunks nf..nf+2 at col 0, nf+3 at D1[384:]
            nc.vector.tensor_tensor(
                PT[:, nf : nf + 3, 0:P],
                PT[:, nf : nf + 3, 0:P],
                maskb[:, None, :].to_broadcast((P, 3, P)),
                MUL,
            )
            nc.vector.tensor_tensor(
                PT[:, nf + 1, 384:512],
                PT[:, nf + 1, 384:512],
                maskb[:],
                MUL,
            )
            for i in range(4):  # tq = 4*qc + i
                tq = nf + i
                av = av_pool.tile([P, 33], F32, tag="av", padded_shape=[P, 512])
                nki = tq + 1
                for ki in range(nki):
                    if ki < nf:
                        lhsT = PT[:, ki, ts(i, P)]
                    elif ki == nf:
                        lhsT = PT[:, nf, ts(i, P)]
                    elif ki == nf + 3:
                        lhsT = PT[:, nf + 1, 384:512]
                    else:  # nf+1 / nf+2 with shifted columns
                        di = ki - nf
                        lhsT = PT[:, nf + di, ts(i - di, P)]
                    nc.tensor.matmul(
                        av[:],
                        lhsT=lhsT,
                        rhs=V[:, ki, g, :],
                        start=(ki == 0),
                        stop=(ki == nki - 1),
                    )
                # AO[q, head-dims] = av[:, 0:32] / Z  (Z = av[:, 32])
                zr = zr_pool.tile([P, 1], F32, tag="zr")
                nc.vector.reciprocal_approx_fast(zr[:], av[:, D : D + 1])
                nc.vector.tensor_scalar_mul(
                    AO[:, i, h // 4, ts(h % 4, D)], av[:, 0:D], zr[:]
                )

        def emit_tail(qc, AO):
            """AO transpose + out-projection + store for one q-chunk."""
            # transpose 16 [128, <=128] blocks (4 tq x 4 quads)
            AOT = ao_pool.tile([P, 4, 4, P], BF16, tag="aot")
            blocks = [(i, qd) for i in range(4) for qd in range(4)]
            for half in range(2):
                tp = sp_pool.tile([P, 12, P], BF16, tag="sp")
                blk = blocks[half * 8 : half * 8 + 8]
                for j, (i, qd) in enumerate(blk):
                    m = P if qd < 3 else 64
                    nc.tensor.transpose(
                        tp[:m, j, :], AO[:, i, qd, 0:m], ident[:]
                    )
                for j, (i, qd) in enumerate(blk):
                    m = P if qd < 3 else 64
                    nc.vector.tensor_copy(AOT[:m, i, qd, :], tp[:m, j, :])
            for i in range(4):
                tg = qc * 4 + i
                ob = ob_pool.tile([P, C], F32, tag="ob")
                po = sp_pool.tile([P, 2, 512], F32, tag="sp")
                for ncol in range(2):
                    for qd in range(4):
                        m = P if qd < 3 else 64
                        nc.tensor.matmul(
                            po[:, ncol, 0:448],
                            lhsT=AOT[:m, i, qd, :],
                            rhs=WoS[:m, qd, ncol * 448 : (ncol + 1) * 448],
                            start=(qd == 0),
                            stop=(qd == 3),
                        )
                    nc.vector.tensor_copy(
                        ob[:, ncol * 448 : (ncol + 1) * 448], po[:, ncol, 0:448]
                    )
                nc.sync.dma_start(ov[:, tg, :], ob[:])

        # software pipeline over (qc, h) pairs
        pairs = [(qc, h) for qc in range(NK) for h in range(HL)]
        prev = None  # (qc, h, PT, AO)
        AO = None
        for qc, h in pairs:
            if h == 0:
                AO = ao_pool.tile([P, 4, 4, P], BF16, tag="ao")
            PT = emit_scores(qc, h)
            if prev is not None:
                pqc, ph, pPT, pAO = prev
                emit_av(pqc, ph, pPT, pAO)
                if ph == HL - 1:
                    emit_tail(pqc, pAO)
            prev = (qc, h, PT, AO)
        pqc, ph, pPT, pAO = prev
        emit_av(pqc, ph, pPT, pAO)
        emit_tail(pqc, pAO)


_NC_CACHE = None


def _build():
    global _NC_CACHE
    if _NC_CACHE is not None:
        return _NC_CACHE
    nc = bacc.Bacc("TRN2", target_bir_lowering=False, debug=False, num_devices=8)
    d = {
        "x": nc.dram_tensor("x", (T, C), mybir.dt.float32r, kind="ExternalInput"),
        "wq": nc.dram_tensor("wq", (C, 512), BF16, kind="ExternalInput"),
        "wk": nc.dram_tensor("wk", (C, P), BF16, kind="ExternalInput"),
        "wv": nc.dram_tensor("wv", (C, P), BF16, kind="ExternalInput"),
        "wo": nc.dram_tensor("wo", (DH, C), BF16, kind="ExternalInput"),
        "ident": nc.dram_tensor("ident", (P, P), BF16, kind="ExternalInput"),
        "identr": nc.dram_tensor(
            "identr", (P, P), mybir.dt.float32r, kind="ExternalInput"
        ),
        "mask": nc.dram_tensor("mask", (P, P), BF16, kind="ExternalInput"),
        "out": nc.dram_tensor("out", (T, C), F32, kind="ExternalOutput"),
    }
    with tile.TileContext(nc) as tc:
        _trace(tc, {k: v[:] for k, v in d.items()})
    nc.compile()
    _NC_CACHE = nc
    return nc


def _in_maps(x, Wq, Wk, Wv, Wo):
    bf = ml_dtypes.bfloat16
    ident = np.eye(P, dtype=np.float32).astype(bf)
    maskb = (
        np.arange(P)[None, :] >= np.arange(P)[:, None]
    ).astype(np.float32).astype(bf)  # [kpos_p, q_j] valid when j >= p
    maps = []
    for c in range(8):
        b, hf = c // 2, c % 2
        heads = HEADS_HALF[hf]
        # Wq columns: slot s, band g <- head 4g+s (missing -> zeros)
        wq = np.zeros((C, 4, P), dtype=np.float32)
        for l, H in enumerate(heads):
            g, s = l // 4, l % 4
            wq[:, s, 32 * g : 32 * g + D] = Wq[:, 32 * H : 32 * H + D]
        kcols = np.concatenate(
            [np.arange(32 * g, 32 * g + D) for g in KV_HALF[hf]]
        )
        hrows = np.concatenate([np.arange(32 * H, 32 * H + D) for H in heads])
        maps.append(
            {
                "x": np.ascontiguousarray(x[b]),
                "wq": np.ascontiguousarray(wq.reshape(C, 512)).astype(bf),
                "wk": np.ascontiguousarray(Wk[:, kcols]).astype(bf),
                "wv": np.ascontiguousarray(Wv[:, kcols]).astype(bf),
                "wo": np.ascontiguousarray(Wo[hrows, :]).astype(bf),
                "ident": ident,
                "identr": np.eye(P, dtype=np.float32),
                "mask": maskb,
            }
        )
    return maps


def run(x, Wq, Wk, Wv, Wo, trace=False):
    nc = _build()
    res = run_bass_kernel_spmd(
        nc, _in_maps(x, Wq, Wk, Wv, Wo), core_ids=list(range(8)), trace=trace
    )
    outs = [r["out"] for r in res.results]
    final = np.empty((4, T, C), np.float32)
    for b in range(4):
        final[b] = outs[2 * b] + outs[2 * b + 1]
    return final, res


def kernel(x, Wq, Wk, Wv, Wo):
    x = np.asarray(x, dtype=np.float32)
    out, _ = run(
        x,
        np.asarray(Wq, np.float32),
        np.asarray(Wk, np.float32),
        np.asarray(Wv, np.float32),
        np.asarray(Wo, np.float32),
    )
    return out
